# revision 26
# baseline (speedup 1.0000x reference)
# MoE (GShard top-1, capacity=S/E) inference kernel for Trainium2, 8 cores.
# Expert-parallel MLP + data-parallel gate.
#
# v3 design (v1 baseline ~501us, v2 ~504us):
#  - gate consumes a host-transposed x shard (xshT); fp32 matmul (min top-2
#    logit gap is 2.3e-5; bf16/tf32 flip argmaxes, each flip ~1.3% rel err).
#  - per-shard local routing (softmax/argmax) before the collective; a single
#    AllGather moves only (expert_id, gate_value) per token (8KB/core).
#    A dummy warm-up AllGather at t=0 absorbs the ncfw dispatch latency.
#  - slot-table inversion via gpsimd local_scatter (per-partition scatter in
#    GPSIMD RAM, negative idx skipped) + 8 PE transposes / ones-matmuls to
#    fold partitions - no DMA scatter descriptors at all (v1: 46us, v2: 70us).
#  - token dispatch via dma_gather(transpose=True) lands rows pre-transposed
#    in the MM1 rhs layout (no dispatch PE transposes).
#  - sync DMA queue order: gate xshT tiles -> w2 full -> w1 ring, so bulk
#    weights stream during the gate/AG/routing phases.
#  - MLP identical to v1 (measured at the sustained-clock floor ~260ns/MM).
import sys

sys.path.insert(0, "/opt/trn_rl_repo")

import numpy as np
import ml_dtypes

import concourse.bass as bass
import concourse.mybir as mybir
import concourse.bacc as bacc
import concourse.tile as tile
from concourse.bass_utils import run_bass_kernel_spmd

S, M, H, E = 8192, 1024, 4096, 8
C = S // E  # 1024 capacity
NCORES = 8
TPC = S // NCORES  # tokens per core shard = 1024
NA = S // 128  # 64 token tiles of 128
NA_LOC = TPC // 128  # 8 local token tiles
F32 = mybir.dt.float32
BF16 = mybir.dt.bfloat16
I16 = mybir.dt.int16
BF = ml_dtypes.bfloat16

X = mybir.AxisListType.X
OP = mybir.AluOpType
ACTF = mybir.ActivationFunctionType


def _build_program():
    nc = bacc.Bacc(
        "TRN2",
        target_bir_lowering=False,
        debug=False,
        num_devices=NCORES,
        dynamic_dma_scratch_size=32768,
        num_swdge_queues=2,
    )

    # ---- I/O ----
    din = {}
    for name, shape, dt in [
        ("xshT", [M, TPC], F32),         # this core's token shard, transposed
        ("xg", [S + 1, M], BF16),        # full x (bf16) + zero row 0, for gather
        ("wg", [M, E], F32),
        ("w1", [M, H], BF16),            # this core's expert inter_w
        ("b1", [H], F32),
        ("w2", [H, M], BF16),            # this core's expert output_w
        ("identf", [128, 128], F32),
        ("tri", [128, 128], F32),        # tri[k,p] = 1 if k < p
        ("ones_k", [128, 1], F32),       # column of ones (partition reduce)
        ("ones_kb", [128, 1], BF16),
        ("ones_p", [1, 128], F32),       # row of ones (partition broadcast)
        ("wperm", [128, 8, 128], F32),   # wrap-16 permutation mats (see host)
        ("iota_e", [128, NA * E], F32),  # tile(0..7) per token slot
        ("tokhi", [128, NA], BF16),      # (t+1) >> 6  (bf16-exact)
        ("toklo", [128, NA], BF16),      # (t+1) & 63  (bf16-exact)
        ("cid", [128, 1], F32),          # this core's index
        ("b2bc", [128, M], F32),         # b2 replicated across partitions
    ]:
        din[name] = nc.dram_tensor(name, shape, dt, kind="ExternalInput")

    out_e = nc.dram_tensor("outE", [C, M], F32, kind="ExternalOutput")
    out_meta = nc.dram_tensor("outmeta", [C, 2], F32, kind="ExternalOutput")

    with tile.TileContext(nc) as tc:
        _kernel_body(nc, tc, din, out_e, out_meta)

    nc.compile()
    return nc


def _kernel_body(nc, tc, din, out_e, out_meta):
    from contextlib import ExitStack

    stack = ExitStack()
    cpool = stack.enter_context(tc.tile_pool(name="const", bufs=1))
    dram = stack.enter_context(tc.tile_pool(name="dram", bufs=1, space="DRAM"))

    def cload(name, shape, dt=F32, src=None):
        t = cpool.tile(shape, dt, tag=name, name=name)
        nc.scalar.dma_start(t[:], src if src is not None else din[name].ap())
        return t

    # Warm up the collectives firmware first (TOPSP dispatch is ~11us cold).
    dumI = dram.tile([1, 64], F32, name="dumI")
    dumO = dram.tile([NCORES, 1, 64], F32, name="dumO")
    dumS = cpool.tile([1, 64], F32, tag="dumS", name="dumS")
    nc.vector.memset(dumS[:], 0.0)
    nc.scalar.dma_start(dumI[:], dumS[:])
    nc.gpsimd.collective_compute(
        "AllGather",
        OP.bypass,
        replica_groups=[list(range(NCORES))],
        ins=[dumI[:]],
        outs=[dumO[:]],
    )

    # consts on the SCALAR queue so the sync queue can stream bulk weights
    # from t=0. Order matters: gate needs wg/identf first.
    wg_sb = cload("wg", [128, M // 128, E], src=din["wg"].ap().rearrange("(kb p) e -> p kb e", p=128))
    identf = cload("identf", [128, 128])
    iota_e = cload("iota_e", [128, NA * E])
    tri = cload("tri", [128, 128])
    ones_k = cload("ones_k", [128, 1])
    ones_kb = cload("ones_kb", [128, 1], BF16)
    ones_p = cload("ones_p", [1, 128])
    wperm = cload("wperm", [128, 8, 128])
    tokhi = cload("tokhi", [128, NA], BF16)
    toklo = cload("toklo", [128, NA], BF16)
    cid = cload("cid", [128, 1])
    b1_sb = cload("b1", [128, H // 128], src=din["b1"].ap().rearrange("(hb p) -> p hb", p=128))
    b2bc = cload("b2bc", [128, M])

    # DRAM scratch for the main collective
    loglocD = dram.tile([128, NA_LOC, 2], F32, name="logloc")
    logfullD = dram.tile([NCORES, 128, NA_LOC, 2], F32, name="logfull")

    mpool = stack.enter_context(tc.tile_pool(name="mlp", bufs=1))
    w2s = [mpool.tile([128, M], BF16, tag=f"w2{hb}", name=f"w2{hb}") for hb in range(H // 128)]

    gstack = ExitStack()
    gpool = gstack.enter_context(tc.tile_pool(name="gate", bufs=1))

    # PE warm-up burst: ~3.5us of back-to-back transposes fires the HAM
    # SHORT window so the gate matmuls run at 2.4GHz instead of 1.2.
    with tc.tile_pool(name="wpsum", bufs=1, space="PSUM") as wpsum:
        wps = wpsum.tile([128, 128], F32, tag="wps", name="wps")
        for _ in range(30):
            nc.tensor.transpose(wps[:], identf[:], identf[:])

    # ================= Phase G: gate logits (fp32) =================
    # logitsT[e, t] = sum_m wg[m, e] * xT[m, t]; accumulate over 8 m-blocks.
    with tc.tile_pool(name="gpsum", bufs=1, space="PSUM") as gpsum:
        lps = [gpsum.tile([E, 512], F32, tag=f"lps{th}", name=f"lps{th}") for th in range(2)]
        for kb in range(M // 128):
            xt = gpool.tile([128, TPC], F32, tag="xt", name="xt", bufs=3)
            nc.sync.dma_start(xt[:], din["xshT"].ap()[kb * 128:(kb + 1) * 128, :])
            for th in range(2):
                nc.tensor.matmul(
                    lps[th][:],
                    wg_sb[:, kb, :],
                    xt[:, th * 512:(th + 1) * 512],
                    start=(kb == 0),
                    stop=(kb == M // 128 - 1),
                )
        ltr = gpool.tile([E, TPC], F32, tag="ltr", name="ltr")
        for th in range(2):
            nc.vector.tensor_copy(ltr[:, th * 512:(th + 1) * 512], lps[th][:])
        # transpose logitsT -> [t%128, a, e]
        tpp = gpsum.tile([128, NA_LOC * E], F32, tag="tpp", name="tpp")
        for a in range(NA_LOC):
            nc.tensor.transpose(
                tpp[:, a * E:(a + 1) * E],
                ltr[:, a * 128:(a + 1) * 128],
                identf[0:E, 0:E],
            )
        L_loc = gpool.tile([128, NA_LOC, E], F32, tag="L_loc", name="L_loc")
        nc.vector.tensor_copy(L_loc[:].rearrange("p a e -> p (a e)"), tpp[:])

    # ============ Phase L: local routing (this shard's tokens) ============
    # softmax without max-subtraction (|logit| < ~5 for this distribution);
    # the argmax one-hot via is_equal(L, max) is exact in fp32.
    mx = gpool.tile([128, NA_LOC], F32, tag="mx", name="mx")
    nc.vector.reduce_max(mx[:], L_loc[:, :, :], axis=X)
    ex = gpool.tile([128, NA_LOC, E], F32, tag="ex", name="ex")
    nc.scalar.activation(ex[:, :, :], L_loc[:, :, :], ACTF.Exp)
    se = gpool.tile([128, NA_LOC], F32, tag="se", name="se")
    nc.vector.reduce_sum(se[:].unsqueeze(2), ex[:, :, :], axis=X)
    emx = gpool.tile([128, NA_LOC], F32, tag="emx", name="emx")
    nc.scalar.activation(emx[:], mx[:], ACTF.Exp)
    rse = gpool.tile([128, NA_LOC], F32, tag="rse", name="rse")
    nc.vector.reciprocal(rse[:], se[:])
    gvl = gpool.tile([128, NA_LOC], F32, tag="gvl", name="gvl")
    nc.vector.tensor_tensor(gvl[:], emx[:], rse[:], op=OP.mult)
    m1 = gpool.tile([128, NA_LOC, E], F32, tag="m1", name="m1")
    mxb = mx[:].unsqueeze(2).broadcast_to([128, NA_LOC, E])
    nc.vector.tensor_tensor(m1[:, :, :], L_loc[:, :, :], mxb, op=OP.is_equal)
    eidm = gpool.tile([128, NA_LOC, E], F32, tag="eidm", name="eidm")
    iota8 = iota_e[:, 0:NA_LOC * E].rearrange("p (a e) -> p a e", e=E)
    nc.vector.tensor_tensor(eidm[:, :, :], iota8, m1[:, :, :], op=OP.mult)
    eid = gpool.tile([128, NA_LOC], F32, tag="eid", name="eid")
    nc.vector.reduce_sum(eid[:].unsqueeze(2), eidm[:, :, :], axis=X)

    payl = gpool.tile([128, NA_LOC, 2], F32, tag="payl", name="payl")
    nc.vector.tensor_copy(payl[:, :, 0:1], eid[:].unsqueeze(2))
    nc.vector.tensor_copy(payl[:, :, 1:2], gvl[:].unsqueeze(2))
    nc.scalar.dma_start(loglocD[:], payl[:, :, :])
    nc.gpsimd.collective_compute(
        "AllGather",
        OP.bypass,
        replica_groups=[list(range(NCORES))],
        ins=[loglocD[:]],
        outs=[logfullD[:]],
    )
    gstack.close()

    # ============ Phase R: global routing (all tokens, redundant) ============
    rstack = ExitStack()
    rpool = rstack.enter_context(tc.tile_pool(name="rt", bufs=1))
    rpsum = rstack.enter_context(tc.tile_pool(name="rpsum", bufs=1, space="PSUM"))

    def rt(tag, shape=(128, NA * E), dt=F32):
        return rpool.tile(list(shape), dt, tag=tag, name=tag)

    LG = rt("LG", (128, NA, 2))
    LG4 = LG[:, :, :].rearrange("p (d a) c -> p d a c", d=NCORES)
    nc.sync.dma_start(
        LG4[:, 0:4, :, :], logfullD[0:4].rearrange("d p a c -> p d a c")
    )
    nc.scalar.dma_start(
        LG4[:, 4:8, :, :], logfullD[4:8].rearrange("d p a c -> p d a c")
    )
    eid_all = LG[:, :, 0:1]
    gv_all = LG[:, :, 1:2]

    # w2 preload on the scalar queue BEHIND the (AG-dependent) LG load: the
    # fabric stays quiet while the collectives firmware boots, then w2's 8MB
    # streams during the routing phase (MM2 needs it only ~100us later).
    for hb in range(H // 128):
        nc.scalar.dma_start(w2s[hb][:], din["w2"].ap()[hb * 128:(hb + 1) * 128, :])

    # second PE warm-up burst (anchored on LG so it lands right after the
    # AG) so the routing matmuls run at full clock; overlaps the DVE chain.
    wps2 = rpsum.tile([128, 128], F32, tag="wps2", name="wps2")
    LGflat = LG[:, :, :].rearrange("p a c -> p (a c)")
    for _ in range(17):
        nc.tensor.transpose(wps2[:], LGflat, identf[:])

    mask = rt("mask")
    mask3 = mask[:].rearrange("p (a e) -> p a e", e=E)
    iota3 = iota_e[:].rearrange("p (a e) -> p a e", e=E)
    nc.vector.tensor_tensor(mask3, iota3, eid_all.broadcast_to([128, NA, E]), op=OP.is_equal)

    # cast the gate column for the slot table while the cumsum runs
    gate_bf = rt("gate_bf", (128, NA), BF16)
    nc.gpsimd.tensor_copy(gate_bf[:].unsqueeze(2), gv_all)
    # ism = is this core's expert
    ism = rt("ism", (128, NA))
    nc.gpsimd.tensor_scalar(ism[:], eid_all.rearrange("p a c -> p (a c)"), cid[:, 0:1], None, op0=OP.is_equal)

    # exclusive cumsum over all tokens: per-tile tri matmul + tile offsets
    totp = rpsum.tile([1, NA * E], F32, tag="totp", name="totp")
    nc.tensor.matmul(totp[:], ones_k[:], mask[:], start=True, stop=True)
    tot = rt("tot", (1, NA * E))
    nc.vector.tensor_copy(tot[:], totp[:])

    cur = tot
    for i, k in enumerate((1, 2, 4, 8, 16, 32)):
        nxt = rt(f"sc{i % 2}", (1, NA * E))
        c3 = cur[:].rearrange("p (a e) -> p a e", e=E)
        n3 = nxt[:].rearrange("p (a e) -> p a e", e=E)
        nc.vector.tensor_copy(n3[:, 0:k, :], c3[:, 0:k, :])
        nc.vector.tensor_tensor(n3[:, k:NA, :], c3[:, k:NA, :], c3[:, 0:NA - k, :], op=OP.add)
        cur = nxt
    exc = rt("exc", (1, NA * E))
    nc.vector.tensor_tensor(exc[:], cur[:], tot[:], op=OP.subtract)

    locp = rpsum.tile([128, NA * E], F32, tag="locp", name="locp")
    nc.tensor.matmul(locp[:], tri[:], mask[:], start=True, stop=False)
    nc.tensor.matmul(locp[:], ones_p[:], exc[:], start=False, stop=True)

    # m1k = mask * (loc < C), reading loc straight from PSUM
    m1k = rt("m1k")
    m1k3 = m1k[:].rearrange("p (a e) -> p a e", e=E)
    nc.vector.scalar_tensor_tensor(
        m1k[:], locp[:], float(C), mask[:], op0=OP.is_lt, op1=OP.mult
    )

    posm = rt("posm")
    nc.vector.tensor_tensor(posm[:], locp[:], m1k[:], op=OP.mult)
    pos = rt("pos", (128, NA))
    nc.vector.reduce_sum(pos[:].unsqueeze(2), posm[:].rearrange("p (a e) -> p a e", e=E), axis=X)
    kept = rt("kept", (128, NA))
    nc.vector.reduce_sum(kept[:].unsqueeze(2), m1k3, axis=X)
    vm = rt("vm", (128, NA))
    nc.gpsimd.tensor_tensor(vm[:], ism[:], kept[:], op=OP.mult)

    # slot index for this core's kept tokens, -1 otherwise:
    # aidx = vm * (pos + 1) - 1
    av = rt("av", (128, NA))
    nc.vector.scalar_tensor_tensor(av[:], pos[:], 1.0, vm[:], op0=OP.add, op1=OP.mult)
    aidx = rt("aidx", (128, NA), I16)
    nc.vector.tensor_scalar(aidx[:], av[:], -1.0, None, op0=OP.add)

    # per-partition scatter into the slot-indexed accumulators (bf16 lanes):
    # Ah[p, c] = (tok+1)>>6 if this p sourced slot c (0 elsewhere), Al = low
    # 6 bits, Ag = gate. The PE then folds the source-partition dim with
    # ones-matmuls: lane[q, j] = sum_p A[p, j*128+q] (exactly one nonzero).
    Ah = rt("Ah", (128, C), BF16)
    nc.gpsimd.local_scatter(Ah[:], tokhi[:], aidx[:], 128, C, NA)
    Al = rt("Al", (128, C), BF16)
    nc.gpsimd.local_scatter(Al[:], toklo[:], aidx[:], 128, C, NA)
    Ag = rt("Ag", (128, C), BF16)
    nc.gpsimd.local_scatter(Ag[:], gate_bf[:], aidx[:], 128, C, NA)

    gps = rpsum.tile([128, 3, C // 128], F32, tag="gps", name="gps")
    for li, A in enumerate((Ah, Al, Ag)):
        A3 = A[:].rearrange("p (j q) -> p j q", q=128)
        for j in range(C // 128):
            nc.tensor.matmul(
                gps[:, li, j:j + 1], A3[:, j, :], ones_kb[:], start=True, stop=True
            )
    gsb = rt("gsb", (128, 3, C // 128))
    nc.vector.tensor_copy(gsb[:, :, :], gps[:, :, :])
    toks = rt("toks", (128, C // 128))
    nc.vector.scalar_tensor_tensor(
        toks[:], gsb[:, 0, :], 64.0, gsb[:, 1, :], op0=OP.mult, op1=OP.add
    )
    gates = cpool.tile([128, C // 128], F32, tag="gates", name="gates")
    nc.vector.tensor_copy(gates[:], gsb[:, 2, :])

    slotm = rt("slotm", (128, C // 128, 2))
    nc.vector.tensor_copy(slotm[:, :, 0:1], toks[:].unsqueeze(2))
    nc.vector.tensor_copy(slotm[:, :, 1:2], gates[:].unsqueeze(2))
    nc.scalar.dma_start(
        out_meta.ap()[:].rearrange("(j p) two -> p j two", p=128),
        slotm[:, :, :],
    )

    # gather idx: wrap-16 shuffle of the slot->token column via perm matmuls
    tk16p = rpsum.tile([128, 8, C // 128], F32, tag="tk16p", name="tk16p")
    for g in range(8):
        nc.tensor.matmul(tk16p[:, g, :], wperm[:, g, :], toks[:], start=True, stop=True)
    tok16 = cpool.tile([128, C // 128, 8], I16, tag="tok16", name="tok16")
    nc.vector.tensor_copy(
        tok16[:, :, :],
        tk16p[:, :, :].rearrange("p g j -> p j g"),
    )

    # gather this expert's token rows pre-transposed (16-bit transpose mode):
    # gxh[h][p, kb, c] = x[tok_c, kb*128 + p] — directly usable as MM1's rhs.
    gxh = [
        cpool.tile([128, M // 128, C // 2], BF16, tag=f"gxh{h}", name=f"gxh{h}")
        for h in range(2)
    ]
    for h in range(2):
        nc.gpsimd.dma_gather(
            gxh[h][:, :, :],
            din["xg"].ap(),
            tok16[:].rearrange("p j g -> p (j g)")[:, h * (C // 32):(h + 1) * (C // 32)],
            C // 2,
            C // 2,
            M,
            transpose=True,
            queue_num=h,
        )

    rstack.close()

    # ================= Phase M: expert MLP =================
    NJ = C // 128  # 8 c-blocks
    NKB = M // 128  # 8 m-blocks
    NHB = H // 128  # 32 h-blocks

    wpool = stack.enter_context(tc.tile_pool(name="wstream", bufs=32))
    opool = stack.enter_context(tc.tile_pool(name="out", bufs=2))
    mpsum = stack.enter_context(tc.tile_pool(name="mpsum", bufs=8, space="PSUM"))

    # MM1: hT[hb][h, c] = gelu(w1.T @ dispxT + b1)
    hts = [mpool.tile([128, C], BF16, tag=f"ht{hb}", name=f"ht{hb}") for hb in range(NHB)]
    for hp in range(NHB // 2):  # 16 rounds of 2 h-blocks
        pss = [[mpsum.tile([128, 512], F32, tag="mmp", name="mmp") for _ in range(2)] for _ in range(2)]
        for kb in range(NKB):
            w1t = wpool.tile([128, 256], BF16, tag="w1t", name="w1t")
            nc.sync.dma_start(
                w1t[:],
                din["w1"].ap()[kb * 128:(kb + 1) * 128, hp * 256:(hp + 1) * 256],
            )
            for h2 in range(2):
                for ch in range(2):
                    nc.tensor.matmul(
                        pss[h2][ch][:],
                        w1t[:, h2 * 128:(h2 + 1) * 128],
                        gxh[ch][:, kb, :],
                        start=(kb == 0),
                        stop=(kb == NKB - 1),
                    )
        for h2 in range(2):
            hb = hp * 2 + h2
            for ch in range(2):
                nc.scalar.activation(
                    hts[hb][:, ch * 512:(ch + 1) * 512],
                    pss[h2][ch][:],
                    ACTF.Gelu,
                    bias=b1_sb[:, hb:hb + 1],
                )

    # MM2: out[c, m] = (hT.T @ w2 + b2) * gate
    for jc in range(NJ):
        ops_ = [mpsum.tile([128, 512], F32, tag="mmp", name="mmp") for _ in range(2)]
        for hb in range(NHB):
            for mh in range(2):
                nc.tensor.matmul(
                    ops_[mh][:],
                    hts[hb][:, jc * 128:(jc + 1) * 128],
                    w2s[hb][:, mh * 512:(mh + 1) * 512],
                    start=(hb == 0),
                    stop=(hb == NHB - 1),
                )
        osb = opool.tile([128, M], F32, tag="osb", name="osb")
        for mh in range(2):
            sl = slice(mh * 512, (mh + 1) * 512)
            nc.vector.tensor_tensor(osb[:, sl], ops_[mh][:], b2bc[:, sl], op=OP.add)
            nc.vector.tensor_scalar(
                osb[:, sl], osb[:, sl], gates[:, jc:jc + 1], None, op0=OP.mult
            )
        nc.sync.dma_start(out_e.ap()[jc * 128:(jc + 1) * 128, :], osb[:])

    stack.close()


_NC_CACHE = {}


def _get_nc():
    if "nc" not in _NC_CACHE:
        _NC_CACHE["nc"] = _build_program()
    return _NC_CACHE["nc"]


def _host_consts():
    t = (np.arange(NA)[None, :] * 128 + np.arange(128)[:, None]).astype(np.int64)
    wperm = np.zeros((128, 8, 128), np.float32)
    for g in range(8):
        for qq in range(128):
            wperm[g * 16 + (qq % 16), g, qq] = 1.0
    return {
        "identf": np.eye(128, dtype=np.float32),
        "tri": (np.arange(128)[:, None] < np.arange(128)[None, :]).astype(np.float32),
        "ones_k": np.ones((128, 1), np.float32),
        "ones_kb": np.ones((128, 1), BF),
        "ones_p": np.ones((1, 128), np.float32),
        "wperm": wperm,
        "iota_e": np.tile(np.arange(E, dtype=np.float32), (128, NA)),
        "tokhi": ((t + 1) >> 6).astype(BF),
        "toklo": ((t + 1) & 63).astype(BF),
    }


def _in_maps(x, wg, inter_w, inter_b, output_w, output_b):
    consts = _host_consts()
    xg = np.concatenate([np.zeros((1, M), np.float32), x]).astype(BF)
    in_maps = []
    for d in range(NCORES):
        in_maps.append(
            {
                "xshT": np.ascontiguousarray(x[d * TPC:(d + 1) * TPC].T),
                "xg": xg,
                "wg": wg,
                "w1": inter_w[d].astype(BF),
                "b1": inter_b[d],
                "w2": output_w[d].astype(BF),
                "b2bc": np.tile(output_b[d], (128, 1)),
                "cid": np.full((128, 1), d, np.float32),
                **consts,
            }
        )
    return in_maps


def kernel(x, wg, inter_w, inter_b, output_w, output_b):
    x = np.asarray(x, np.float32)
    wg = np.asarray(wg, np.float32)
    inter_w = np.asarray(inter_w, np.float32)
    inter_b = np.asarray(inter_b, np.float32)
    output_w = np.asarray(output_w, np.float32)
    output_b = np.asarray(output_b, np.float32)

    nc = _get_nc()
    res = run_bass_kernel_spmd(
        nc, _in_maps(x, wg, inter_w, inter_b, output_w, output_b), list(range(NCORES))
    )

    y = np.zeros((S, M), np.float32)
    for d in range(NCORES):
        meta = res.results[d]["outmeta"]
        oute = res.results[d]["outE"]
        tok1 = np.rint(meta[:, 0]).astype(np.int64)
        valid = tok1 > 0
        y[tok1[valid] - 1] = oute[valid]
    return y


if __name__ == "__main__":
    pass


# revision 27
# speedup vs baseline: 1.0626x; 1.0626x over previous
# MoE (GShard top-1, capacity=S/E) inference kernel for Trainium2, 8 cores.
# Expert-parallel MLP + data-parallel gate.
#
# v3 design (v1 baseline ~501us, v2 ~504us):
#  - gate consumes a host-transposed x shard (xshT); fp32 matmul (min top-2
#    logit gap is 2.3e-5; bf16/tf32 flip argmaxes, each flip ~1.3% rel err).
#  - per-shard local routing (softmax/argmax) before the collective; a single
#    AllGather moves only (expert_id, gate_value) per token (8KB/core).
#    A dummy warm-up AllGather at t=0 absorbs the ncfw dispatch latency.
#  - slot-table inversion via gpsimd local_scatter (per-partition scatter in
#    GPSIMD RAM, negative idx skipped) + 8 PE transposes / ones-matmuls to
#    fold partitions - no DMA scatter descriptors at all (v1: 46us, v2: 70us).
#  - token dispatch via dma_gather(transpose=True) lands rows pre-transposed
#    in the MM1 rhs layout (no dispatch PE transposes).
#  - sync DMA queue order: gate xshT tiles -> w2 full -> w1 ring, so bulk
#    weights stream during the gate/AG/routing phases.
#  - MLP identical to v1 (measured at the sustained-clock floor ~260ns/MM).
import sys

sys.path.insert(0, "/opt/trn_rl_repo")

import numpy as np
import ml_dtypes

import concourse.bass as bass
import concourse.mybir as mybir
import concourse.bacc as bacc
import concourse.tile as tile
from concourse.bass_utils import run_bass_kernel_spmd

S, M, H, E = 8192, 1024, 4096, 8
C = S // E  # 1024 capacity
NCORES = 8
TPC = S // NCORES  # tokens per core shard = 1024
NA = S // 128  # 64 token tiles of 128
NA_LOC = TPC // 128  # 8 local token tiles
F32 = mybir.dt.float32
BF16 = mybir.dt.bfloat16
I16 = mybir.dt.int16
BF = ml_dtypes.bfloat16

X = mybir.AxisListType.X
OP = mybir.AluOpType
ACTF = mybir.ActivationFunctionType


def _build_program():
    nc = bacc.Bacc(
        "TRN2",
        target_bir_lowering=False,
        debug=False,
        num_devices=NCORES,
        dynamic_dma_scratch_size=32768,
        num_swdge_queues=2,
    )

    # ---- I/O ----
    din = {}
    for name, shape, dt in [
        ("xshT", [M, TPC], F32),         # this core's token shard, transposed
        ("xg", [S + 1, M], BF16),        # full x (bf16) + zero row 0, for gather
        ("wg", [M, E], F32),
        ("w1", [M, H], BF16),            # this core's expert inter_w
        ("b1", [H], F32),
        ("w2", [H, M], BF16),            # this core's expert output_w
        ("identf", [128, 128], F32),
        ("tri", [128, 128], F32),        # tri[k,p] = 1 if k < p
        ("ones_k", [128, 1], F32),       # column of ones (partition reduce)
        ("ones_kb", [128, 1], BF16),
        ("ones_p", [1, 128], F32),       # row of ones (partition broadcast)
        ("wperm", [128, 8, 128], F32),   # wrap-16 permutation mats (see host)
        ("iota_e", [128, NA * E], F32),  # tile(0..7) per token slot
        ("tokhi", [128, NA], BF16),      # (t+1) >> 6  (bf16-exact)
        ("toklo", [128, NA], BF16),      # (t+1) & 63  (bf16-exact)
        ("cid", [128, 1], F32),          # this core's index
        ("b2bc", [128, M], F32),         # b2 replicated across partitions
    ]:
        din[name] = nc.dram_tensor(name, shape, dt, kind="ExternalInput")

    out_e = nc.dram_tensor("outE", [C, M], F32, kind="ExternalOutput")
    out_meta = nc.dram_tensor("outmeta", [C, 2], F32, kind="ExternalOutput")

    with tile.TileContext(nc) as tc:
        _kernel_body(nc, tc, din, out_e, out_meta)

    nc.compile()
    return nc


def _kernel_body(nc, tc, din, out_e, out_meta):
    from contextlib import ExitStack

    stack = ExitStack()
    cpool = stack.enter_context(tc.tile_pool(name="const", bufs=1))
    dram = stack.enter_context(tc.tile_pool(name="dram", bufs=1, space="DRAM"))

    def cload(name, shape, dt=F32, src=None):
        t = cpool.tile(shape, dt, tag=name, name=name)
        nc.scalar.dma_start(t[:], src if src is not None else din[name].ap())
        return t

    # Warm up the collectives firmware first (TOPSP dispatch is ~11us cold).
    dumI = dram.tile([1, 64], F32, name="dumI")
    dumO = dram.tile([NCORES, 1, 64], F32, name="dumO")
    dumS = cpool.tile([1, 64], F32, tag="dumS", name="dumS")
    nc.vector.memset(dumS[:], 0.0)
    nc.scalar.dma_start(dumI[:], dumS[:])
    nc.gpsimd.collective_compute(
        "AllGather",
        OP.bypass,
        replica_groups=[list(range(NCORES))],
        ins=[dumI[:]],
        outs=[dumO[:]],
    )

    # consts on the SCALAR queue so the sync queue can stream bulk weights
    # from t=0. Order matters: gate needs wg/identf first.
    wg_sb = cload("wg", [128, M // 128, E], src=din["wg"].ap().rearrange("(kb p) e -> p kb e", p=128))
    identf = cload("identf", [128, 128])
    iota_e = cload("iota_e", [128, NA * E])
    tri = cload("tri", [128, 128])
    ones_k = cload("ones_k", [128, 1])
    ones_kb = cload("ones_kb", [128, 1], BF16)
    ones_p = cload("ones_p", [1, 128])
    wperm = cload("wperm", [128, 8, 128])
    tokhi = cload("tokhi", [128, NA], BF16)
    toklo = cload("toklo", [128, NA], BF16)
    cid = cload("cid", [128, 1])
    b1_sb = cload("b1", [128, H // 128], src=din["b1"].ap().rearrange("(hb p) -> p hb", p=128))
    b2bc = cload("b2bc", [128, M])

    # DRAM scratch for the main collective
    loglocD = dram.tile([128, NA_LOC, 2], F32, name="logloc")
    logfullD = dram.tile([NCORES, 128, NA_LOC, 2], F32, name="logfull")

    mpool = stack.enter_context(tc.tile_pool(name="mlp", bufs=1))
    w2s = [mpool.tile([128, M], BF16, tag=f"w2{hb}", name=f"w2{hb}") for hb in range(H // 128)]

    gstack = ExitStack()
    gpool = gstack.enter_context(tc.tile_pool(name="gate", bufs=1))

    # PE warm-up burst: ~3.5us of back-to-back transposes fires the HAM
    # SHORT window so the gate matmuls run at 2.4GHz instead of 1.2.
    with tc.tile_pool(name="wpsum", bufs=1, space="PSUM") as wpsum:
        wps = wpsum.tile([128, 128], F32, tag="wps", name="wps")
        for _ in range(30):
            nc.tensor.transpose(wps[:], identf[:], identf[:])

    # ================= Phase G: gate logits (fp32) =================
    # logitsT[e, t] = sum_m wg[m, e] * xT[m, t]; accumulate over 8 m-blocks.
    with tc.tile_pool(name="gpsum", bufs=1, space="PSUM") as gpsum:
        lps = [gpsum.tile([E, 512], F32, tag=f"lps{th}", name=f"lps{th}") for th in range(2)]
        for kb in range(M // 128):
            xt = gpool.tile([128, TPC], F32, tag="xt", name="xt", bufs=3)
            nc.sync.dma_start(xt[:], din["xshT"].ap()[kb * 128:(kb + 1) * 128, :])
            for th in range(2):
                nc.tensor.matmul(
                    lps[th][:],
                    wg_sb[:, kb, :],
                    xt[:, th * 512:(th + 1) * 512],
                    start=(kb == 0),
                    stop=(kb == M // 128 - 1),
                )
        ltr = gpool.tile([E, TPC], F32, tag="ltr", name="ltr")
        for th in range(2):
            nc.vector.tensor_copy(ltr[:, th * 512:(th + 1) * 512], lps[th][:])
        # transpose logitsT -> [t%128, a, e]
        tpp = gpsum.tile([128, NA_LOC * E], F32, tag="tpp", name="tpp")
        for a in range(NA_LOC):
            nc.tensor.transpose(
                tpp[:, a * E:(a + 1) * E],
                ltr[:, a * 128:(a + 1) * 128],
                identf[0:E, 0:E],
            )
        L_loc = gpool.tile([128, NA_LOC, E], F32, tag="L_loc", name="L_loc")
        nc.vector.tensor_copy(L_loc[:].rearrange("p a e -> p (a e)"), tpp[:])

    # ============ Phase L: local routing (this shard's tokens) ============
    # softmax without max-subtraction (|logit| < ~5 for this distribution);
    # the argmax one-hot via is_equal(L, max) is exact in fp32.
    mx = gpool.tile([128, NA_LOC], F32, tag="mx", name="mx")
    nc.vector.reduce_max(mx[:], L_loc[:, :, :], axis=X)
    ex = gpool.tile([128, NA_LOC, E], F32, tag="ex", name="ex")
    nc.scalar.activation(ex[:, :, :], L_loc[:, :, :], ACTF.Exp)
    se = gpool.tile([128, NA_LOC], F32, tag="se", name="se")
    nc.vector.reduce_sum(se[:].unsqueeze(2), ex[:, :, :], axis=X)
    emx = gpool.tile([128, NA_LOC], F32, tag="emx", name="emx")
    nc.scalar.activation(emx[:], mx[:], ACTF.Exp)
    rse = gpool.tile([128, NA_LOC], F32, tag="rse", name="rse")
    nc.vector.reciprocal(rse[:], se[:])
    gvl = gpool.tile([128, NA_LOC], F32, tag="gvl", name="gvl")
    nc.vector.tensor_tensor(gvl[:], emx[:], rse[:], op=OP.mult)
    m1 = gpool.tile([128, NA_LOC, E], F32, tag="m1", name="m1")
    mxb = mx[:].unsqueeze(2).broadcast_to([128, NA_LOC, E])
    nc.vector.tensor_tensor(m1[:, :, :], L_loc[:, :, :], mxb, op=OP.is_equal)
    eidm = gpool.tile([128, NA_LOC, E], F32, tag="eidm", name="eidm")
    iota8 = iota_e[:, 0:NA_LOC * E].rearrange("p (a e) -> p a e", e=E)
    nc.vector.tensor_tensor(eidm[:, :, :], iota8, m1[:, :, :], op=OP.mult)
    eid = gpool.tile([128, NA_LOC], F32, tag="eid", name="eid")
    nc.vector.reduce_sum(eid[:].unsqueeze(2), eidm[:, :, :], axis=X)

    payl = gpool.tile([128, NA_LOC, 2], F32, tag="payl", name="payl")
    nc.vector.tensor_copy(payl[:, :, 0:1], eid[:].unsqueeze(2))
    nc.vector.tensor_copy(payl[:, :, 1:2], gvl[:].unsqueeze(2))
    nc.scalar.dma_start(loglocD[:], payl[:, :, :])
    nc.gpsimd.collective_compute(
        "AllGather",
        OP.bypass,
        replica_groups=[list(range(NCORES))],
        ins=[loglocD[:]],
        outs=[logfullD[:]],
    )
    gstack.close()

    # ============ Phase R: global routing (all tokens, redundant) ============
    rstack = ExitStack()
    rpool = rstack.enter_context(tc.tile_pool(name="rt", bufs=1))
    rpsum = rstack.enter_context(tc.tile_pool(name="rpsum", bufs=1, space="PSUM"))

    def rt(tag, shape=(128, NA * E), dt=F32):
        return rpool.tile(list(shape), dt, tag=tag, name=tag)

    LG = rt("LG", (128, NA, 2))
    LG4 = LG[:, :, :].rearrange("p (d a) c -> p d a c", d=NCORES)
    nc.scalar.dma_start(
        LG4[:, 0:4, :, :], logfullD[0:4].rearrange("d p a c -> p d a c")
    )
    nc.scalar.dma_start(
        LG4[:, 4:8, :, :], logfullD[4:8].rearrange("d p a c -> p d a c")
    )
    eid_all = LG[:, :, 0:1]
    gv_all = LG[:, :, 1:2]

    # w2 preload, data-dependency-held until the AG completes: a tiny
    # scalar-engine write into each w2 tile reads LG, so the 8MB stream
    # cannot start while the collectives firmware boots (DMA congestion
    # stretches its dispatch from ~14us to ~60us). MM2 needs w2 much later.
    for hb in range(H // 128):
        nc.scalar.activation(w2s[hb][0:1, 0:1], LG[0:1, 0:1, 0:1], ACTF.Identity)
        nc.scalar.dma_start(w2s[hb][:], din["w2"].ap()[hb * 128:(hb + 1) * 128, :])

    # second PE warm-up burst (anchored on LG so it lands right after the
    # AG) so the routing matmuls run at full clock; overlaps the DVE chain.
    wps2 = rpsum.tile([128, 128], F32, tag="wps2", name="wps2")
    LGflat = LG[:, :, :].rearrange("p a c -> p (a c)")
    for _ in range(17):
        nc.tensor.transpose(wps2[:], LGflat, identf[:])

    mask = rt("mask")
    mask3 = mask[:].rearrange("p (a e) -> p a e", e=E)
    iota3 = iota_e[:].rearrange("p (a e) -> p a e", e=E)
    nc.vector.tensor_tensor(mask3, iota3, eid_all.broadcast_to([128, NA, E]), op=OP.is_equal)

    # cast the gate column for the slot table while the cumsum runs
    gate_bf = rt("gate_bf", (128, NA), BF16)
    nc.gpsimd.tensor_copy(gate_bf[:].unsqueeze(2), gv_all)
    # ism = is this core's expert
    ism = rt("ism", (128, NA))
    nc.gpsimd.tensor_scalar(ism[:], eid_all.rearrange("p a c -> p (a c)"), cid[:, 0:1], None, op0=OP.is_equal)

    # exclusive cumsum over all tokens: per-tile tri matmul + tile offsets
    totp = rpsum.tile([1, NA * E], F32, tag="totp", name="totp")
    nc.tensor.matmul(totp[:], ones_k[:], mask[:], start=True, stop=True)
    tot = rt("tot", (1, NA * E))
    nc.vector.tensor_copy(tot[:], totp[:])

    cur = tot
    for i, k in enumerate((1, 2, 4, 8, 16, 32)):
        nxt = rt(f"sc{i % 2}", (1, NA * E))
        c3 = cur[:].rearrange("p (a e) -> p a e", e=E)
        n3 = nxt[:].rearrange("p (a e) -> p a e", e=E)
        nc.vector.tensor_copy(n3[:, 0:k, :], c3[:, 0:k, :])
        nc.vector.tensor_tensor(n3[:, k:NA, :], c3[:, k:NA, :], c3[:, 0:NA - k, :], op=OP.add)
        cur = nxt
    exc = rt("exc", (1, NA * E))
    nc.vector.tensor_tensor(exc[:], cur[:], tot[:], op=OP.subtract)

    locp = rpsum.tile([128, NA * E], F32, tag="locp", name="locp")
    nc.tensor.matmul(locp[:], tri[:], mask[:], start=True, stop=False)
    nc.tensor.matmul(locp[:], ones_p[:], exc[:], start=False, stop=True)

    # m1k = mask * (loc < C), reading loc straight from PSUM
    m1k = rt("m1k")
    m1k3 = m1k[:].rearrange("p (a e) -> p a e", e=E)
    nc.vector.scalar_tensor_tensor(
        m1k[:], locp[:], float(C), mask[:], op0=OP.is_lt, op1=OP.mult
    )

    posm = rt("posm")
    nc.vector.tensor_tensor(posm[:], locp[:], m1k[:], op=OP.mult)
    pos = rt("pos", (128, NA))
    nc.vector.reduce_sum(pos[:].unsqueeze(2), posm[:].rearrange("p (a e) -> p a e", e=E), axis=X)
    kept = rt("kept", (128, NA))
    nc.vector.reduce_sum(kept[:].unsqueeze(2), m1k3, axis=X)
    vm = rt("vm", (128, NA))
    nc.gpsimd.tensor_tensor(vm[:], ism[:], kept[:], op=OP.mult)

    # slot index for this core's kept tokens, -1 otherwise:
    # aidx = vm * (pos + 1) - 1
    av = rt("av", (128, NA))
    nc.vector.scalar_tensor_tensor(av[:], pos[:], 1.0, vm[:], op0=OP.add, op1=OP.mult)
    aidx = rt("aidx", (128, NA), I16)
    nc.vector.tensor_scalar(aidx[:], av[:], -1.0, None, op0=OP.add)

    # per-partition scatter into the slot-indexed accumulators (bf16 lanes):
    # Ah[p, c] = (tok+1)>>6 if this p sourced slot c (0 elsewhere), Al = low
    # 6 bits, Ag = gate. The PE then folds the source-partition dim with
    # ones-matmuls: lane[q, j] = sum_p A[p, j*128+q] (exactly one nonzero).
    Ah = rt("Ah", (128, C), BF16)
    nc.gpsimd.local_scatter(Ah[:], tokhi[:], aidx[:], 128, C, NA)
    Al = rt("Al", (128, C), BF16)
    nc.gpsimd.local_scatter(Al[:], toklo[:], aidx[:], 128, C, NA)
    Ag = rt("Ag", (128, C), BF16)
    nc.gpsimd.local_scatter(Ag[:], gate_bf[:], aidx[:], 128, C, NA)

    gps = rpsum.tile([128, 3, C // 128], F32, tag="gps", name="gps")
    for li, A in enumerate((Ah, Al, Ag)):
        A3 = A[:].rearrange("p (j q) -> p j q", q=128)
        for j in range(C // 128):
            nc.tensor.matmul(
                gps[:, li, j:j + 1], A3[:, j, :], ones_kb[:], start=True, stop=True
            )
    gsb = rt("gsb", (128, 3, C // 128))
    nc.vector.tensor_copy(gsb[:, :, :], gps[:, :, :])
    toks = rt("toks", (128, C // 128))
    nc.vector.scalar_tensor_tensor(
        toks[:], gsb[:, 0, :], 64.0, gsb[:, 1, :], op0=OP.mult, op1=OP.add
    )
    gates = cpool.tile([128, C // 128], F32, tag="gates", name="gates")
    nc.vector.tensor_copy(gates[:], gsb[:, 2, :])

    slotm = rt("slotm", (128, C // 128, 2))
    nc.vector.tensor_copy(slotm[:, :, 0:1], toks[:].unsqueeze(2))
    nc.vector.tensor_copy(slotm[:, :, 1:2], gates[:].unsqueeze(2))
    nc.scalar.dma_start(
        out_meta.ap()[:].rearrange("(j p) two -> p j two", p=128),
        slotm[:, :, :],
    )

    # gather idx: wrap-16 shuffle of the slot->token column via perm matmuls
    tk16p = rpsum.tile([128, 8, C // 128], F32, tag="tk16p", name="tk16p")
    for g in range(8):
        nc.tensor.matmul(tk16p[:, g, :], wperm[:, g, :], toks[:], start=True, stop=True)
    tok16 = cpool.tile([128, C // 128, 8], I16, tag="tok16", name="tok16")
    nc.vector.tensor_copy(
        tok16[:, :, :],
        tk16p[:, :, :].rearrange("p g j -> p j g"),
    )

    # gather this expert's token rows pre-transposed (16-bit transpose mode):
    # gxh[h][p, kb, c] = x[tok_c, kb*128 + p] — directly usable as MM1's rhs.
    gxh = [
        cpool.tile([128, M // 128, C // 2], BF16, tag=f"gxh{h}", name=f"gxh{h}")
        for h in range(2)
    ]
    for h in range(2):
        nc.gpsimd.dma_gather(
            gxh[h][:, :, :],
            din["xg"].ap(),
            tok16[:].rearrange("p j g -> p (j g)")[:, h * (C // 32):(h + 1) * (C // 32)],
            C // 2,
            C // 2,
            M,
            transpose=True,
            queue_num=h,
        )

    rstack.close()

    # ================= Phase M: expert MLP =================
    NJ = C // 128  # 8 c-blocks
    NKB = M // 128  # 8 m-blocks
    NHB = H // 128  # 32 h-blocks

    wpool = stack.enter_context(tc.tile_pool(name="wstream", bufs=32))
    opool = stack.enter_context(tc.tile_pool(name="out", bufs=2))
    mpsum = stack.enter_context(tc.tile_pool(name="mpsum", bufs=8, space="PSUM"))

    # MM1: hT[hb][h, c] = gelu(w1.T @ dispxT + b1)
    hts = [mpool.tile([128, C], BF16, tag=f"ht{hb}", name=f"ht{hb}") for hb in range(NHB)]
    for hp in range(NHB // 2):  # 16 rounds of 2 h-blocks
        pss = [[mpsum.tile([128, 512], F32, tag="mmp", name="mmp") for _ in range(2)] for _ in range(2)]
        for kb in range(NKB):
            w1t = wpool.tile([128, 256], BF16, tag="w1t", name="w1t")
            nc.sync.dma_start(
                w1t[:],
                din["w1"].ap()[kb * 128:(kb + 1) * 128, hp * 256:(hp + 1) * 256],
            )
            for h2 in range(2):
                for ch in range(2):
                    nc.tensor.matmul(
                        pss[h2][ch][:],
                        w1t[:, h2 * 128:(h2 + 1) * 128],
                        gxh[ch][:, kb, :],
                        start=(kb == 0),
                        stop=(kb == NKB - 1),
                    )
        for h2 in range(2):
            hb = hp * 2 + h2
            for ch in range(2):
                nc.scalar.activation(
                    hts[hb][:, ch * 512:(ch + 1) * 512],
                    pss[h2][ch][:],
                    ACTF.Gelu,
                    bias=b1_sb[:, hb:hb + 1],
                )

    # MM2: out[c, m] = (hT.T @ w2 + b2) * gate
    for jc in range(NJ):
        ops_ = [mpsum.tile([128, 512], F32, tag="mmp", name="mmp") for _ in range(2)]
        for hb in range(NHB):
            for mh in range(2):
                nc.tensor.matmul(
                    ops_[mh][:],
                    hts[hb][:, jc * 128:(jc + 1) * 128],
                    w2s[hb][:, mh * 512:(mh + 1) * 512],
                    start=(hb == 0),
                    stop=(hb == NHB - 1),
                )
        osb = opool.tile([128, M], F32, tag="osb", name="osb")
        for mh in range(2):
            sl = slice(mh * 512, (mh + 1) * 512)
            nc.vector.tensor_tensor(osb[:, sl], ops_[mh][:], b2bc[:, sl], op=OP.add)
            nc.vector.tensor_scalar(
                osb[:, sl], osb[:, sl], gates[:, jc:jc + 1], None, op0=OP.mult
            )
        nc.sync.dma_start(out_e.ap()[jc * 128:(jc + 1) * 128, :], osb[:])

    stack.close()


_NC_CACHE = {}


def _get_nc():
    if "nc" not in _NC_CACHE:
        _NC_CACHE["nc"] = _build_program()
    return _NC_CACHE["nc"]


def _host_consts():
    t = (np.arange(NA)[None, :] * 128 + np.arange(128)[:, None]).astype(np.int64)
    wperm = np.zeros((128, 8, 128), np.float32)
    for g in range(8):
        for qq in range(128):
            wperm[g * 16 + (qq % 16), g, qq] = 1.0
    return {
        "identf": np.eye(128, dtype=np.float32),
        "tri": (np.arange(128)[:, None] < np.arange(128)[None, :]).astype(np.float32),
        "ones_k": np.ones((128, 1), np.float32),
        "ones_kb": np.ones((128, 1), BF),
        "ones_p": np.ones((1, 128), np.float32),
        "wperm": wperm,
        "iota_e": np.tile(np.arange(E, dtype=np.float32), (128, NA)),
        "tokhi": ((t + 1) >> 6).astype(BF),
        "toklo": ((t + 1) & 63).astype(BF),
    }


def _in_maps(x, wg, inter_w, inter_b, output_w, output_b):
    consts = _host_consts()
    xg = np.concatenate([np.zeros((1, M), np.float32), x]).astype(BF)
    in_maps = []
    for d in range(NCORES):
        in_maps.append(
            {
                "xshT": np.ascontiguousarray(x[d * TPC:(d + 1) * TPC].T),
                "xg": xg,
                "wg": wg,
                "w1": inter_w[d].astype(BF),
                "b1": inter_b[d],
                "w2": output_w[d].astype(BF),
                "b2bc": np.tile(output_b[d], (128, 1)),
                "cid": np.full((128, 1), d, np.float32),
                **consts,
            }
        )
    return in_maps


def kernel(x, wg, inter_w, inter_b, output_w, output_b):
    x = np.asarray(x, np.float32)
    wg = np.asarray(wg, np.float32)
    inter_w = np.asarray(inter_w, np.float32)
    inter_b = np.asarray(inter_b, np.float32)
    output_w = np.asarray(output_w, np.float32)
    output_b = np.asarray(output_b, np.float32)

    nc = _get_nc()
    res = run_bass_kernel_spmd(
        nc, _in_maps(x, wg, inter_w, inter_b, output_w, output_b), list(range(NCORES))
    )

    y = np.zeros((S, M), np.float32)
    for d in range(NCORES):
        meta = res.results[d]["outmeta"]
        oute = res.results[d]["outE"]
        tok1 = np.rint(meta[:, 0]).astype(np.int64)
        valid = tok1 > 0
        y[tok1[valid] - 1] = oute[valid]
    return y


if __name__ == "__main__":
    pass
